# revision 12
# baseline (speedup 1.0000x reference)
"""ColAttention TRN2 kernel: 8-core data-parallel over batch (2 batches/core).

Math (per batch b, width-column w):
  Q = Wq@x+bq; K = Wk@x+bk; V = Wv@x+bv        (1x1 convs over c)
  S[h,g] = sum_q Q[q,h]K[q,g]; attn = softmax_g(S)
  out = gamma * (V @ attn^T) + x

Device pipeline (all matmuls bf16, fp32 PSUM accumulation):
  host folds bv/gamma*bv via e = gamma*(I+gamma*Wv)^-1 bv:  xb = x+e,
  bq' = bq-Wq@e, bk' = bk-Wk@e  =>  device never touches bv and the
  residual add of xb reproduces the reference exactly (algebra in notes).
  A : Q,K projections (batched over h*w, N=384 chunks)
  B1: per column: S^T[g,h] via MM(lhsT=K_col, rhs=Q_col); exp via ACT
      (no max-subtraction: |S|<~50 so exp stays finite in f32);
      colsum^T[h] via MM(lhsT=expS_col, rhs=ones); 1/colsum on DVE;
      V^T[g,c] via MM(lhsT=x_col, rhs=Wv^T)
  B2: transpose recip^T[h,w-half] -> recipW[w,h] on PE
  B3: bcast tile gamma*r[h] over 128 partitions via K=1 outer-product MM;
      U[c,h] = MM(lhsT=V^T, rhs=expS); final = U*bcast + xb; DMA out.
"""
import sys

sys.path.insert(0, "/opt/trn_rl_repo")

import numpy as np
import ml_dtypes

import concourse.bass as bass
import concourse.bacc as bacc
import concourse.mybir as mybir
import concourse.tile as tile
from concourse.bass_utils import run_bass_kernel_spmd

F32 = mybir.dt.float32
BF16 = mybir.dt.bfloat16
AF = mybir.ActivationFunctionType

P = 128
H = 96          # height = attention sequence length
W = 96          # width  = independent columns
HW = H * W
B_LOC = 2       # batches per core
WH = 48         # columns per w-half
WC = 4          # columns per B-chunk
NCH = WH // WC  # 12 chunks per w-half

# engines for the Q/K projection evacuation, round-robined
_QK_EVAC = ("vector",)
# engine for the final residual add
TTADD_ENGINE = "vector"


def _build():
    nc = bacc.Bacc("TRN2", target_bir_lowering=False, debug=False)

    xb_d = nc.dram_tensor("xb", [B_LOC, 2, P, HW], BF16, kind="ExternalInput")
    cb_d = nc.dram_tensor("cblob", [P, 866], BF16, kind="ExternalInput")
    bb_d = nc.dram_tensor("bblob", [P, 1], F32, kind="ExternalInput")
    out_d = nc.dram_tensor("out", [B_LOC, 2, P, HW], F32, kind="ExternalOutput")

    with tile.TileContext(nc) as tc:
        import contextlib

        ctx = contextlib.ExitStack()
        with ctx:
            consts = ctx.enter_context(tc.tile_pool(name="consts", bufs=1))
            xp = ctx.enter_context(tc.tile_pool(name="xp", bufs=1))
            qkp = ctx.enter_context(tc.tile_pool(name="qkp", bufs=2))
            esp = ctx.enter_context(tc.tile_pool(name="esp", bufs=2))
            vtp = ctx.enter_context(tc.tile_pool(name="vtp", bufs=1))
            rtp = ctx.enter_context(tc.tile_pool(name="rtp", bufs=2))
            fp = ctx.enter_context(tc.tile_pool(name="fp", bufs=1))
            bctp = ctx.enter_context(tc.tile_pool(name="bctp", bufs=3))
            ttp = ctx.enter_context(tc.tile_pool(name="ttp", bufs=3))
            ps = ctx.enter_context(tc.tile_pool(name="ps", bufs=2, space="PSUM"))

            cb_t = consts.tile([P, 866], BF16)
            bb_t = consts.tile([P, 1], F32)
            nc.sync.dma_start(out=cb_t, in_=cb_d.ap())
            nc.sync.dma_start(out=bb_t, in_=bb_d.ap())
            # observers: funnel DMA deps into one engine each (this walrus
            # accepts a single semaphore wait per instruction)
            nc.tensor.ldweights(cb_t[:, 0:128])
            bias_t = consts.tile([P, 1], F32)
            nc.vector.tensor_copy(bias_t, bb_t)
            wq_t = cb_t[:, 0:128].rearrange("p (c m) -> p c m", c=2)
            wk_t = cb_t[:, 128:256].rearrange("p (c m) -> p c m", c=2)
            wvt_t = cb_t[:, 256:768].rearrange("p (c m) -> p c m", c=2)
            bq_t = bias_t[0:64, :]
            bk_t = bias_t[64:128, :]
            invg_t = cb_t[0:H, 769:770]
            idb_t = cb_t[0:H, 770:866]

            for b in range(B_LOC):
                x_t = xp.tile([P, 2, HW], BF16, tag="x")
                for ci in range(2):
                    nc.sync.dma_start(out=x_t[:, ci, :], in_=xb_d.ap()[b, ci])
                    xobs = ttp.tile([1, 2], BF16, tag="xobs", name="xobs")
                    nc.vector.tensor_copy(xobs, x_t[0:1, ci, 0:2])
                # per-(h,w)-column views of x: [128, w(96), h(96)]
                x_cols = [
                    x_t[:, ci, :].rearrange("p (h w) -> p w h", w=W) for ci in range(2)
                ]
                x_rows = [
                    x_t[:, ci, :].rearrange("p (h w) -> p h w", w=W) for ci in range(2)
                ]
                f_ts = [fp.tile([P, HW], F32, tag=f"f{ci}", name=f"f{ci}") for ci in range(2)]
                for f in f_ts:
                    nc.vector.memset(f[0:1, 0:1], 0.0)
                f_cols = [f.rearrange("p (h w) -> p w h", w=W) for f in f_ts]

                for half in range(2):
                    # ---- A: Q/K projections for this w-half -------------------
                    q_t = qkp.tile([64, WH * H], BF16, tag="q")
                    k_t = qkp.tile([64, WH * H], BF16, tag="k")
                    ei = 0
                    for (w_l, b_l, o_t) in ((wq_t, bq_t, q_t), (wk_t, bk_t, k_t)):
                        for hc in range(12):  # 8 h-rows x 48 cols = N=384
                            pr = ps.tile([64, 384], F32, tag="s", bufs=1)
                            for ci in range(2):
                                rhs = x_rows[ci][
                                    :, hc * 8 : (hc + 1) * 8, half * WH : (half + 1) * WH
                                ]
                                nc.tensor.matmul(
                                    pr, w_l[:, ci, :], rhs,
                                    start=(ci == 0), stop=(ci == 1),
                                )
                            dst = o_t[:, hc * 384 : (hc + 1) * 384]
                            if _QK_EVAC[ei % len(_QK_EVAC)] == "act":
                                nc.scalar.activation(
                                    out=dst, in_=pr, func=AF.Identity, bias=b_l, scale=1.0
                                )
                            else:
                                nc.vector.tensor_scalar(
                                    out=dst, in0=pr, scalar1=b_l, scalar2=None,
                                    op0=mybir.AluOpType.add,
                                )
                            ei += 1
                    q_cols = q_t.rearrange("p (h w) -> p w h", w=WH)
                    k_cols = k_t.rearrange("p (h w) -> p w h", w=WH)

                    # ---- B1: scores/exp/colsum/recip + V^T --------------------
                    es_t = esp.tile([H, WH * H], BF16, tag="es")
                    vt_t = vtp.tile([H, WH, 256], BF16, tag="vt")
                    rt_t = rtp.tile([H, WH], F32, tag="rt")
                    for ch in range(NCH):
                        s_t = ps.tile([H, 512], F32, tag="s", bufs=1)
                        s3 = s_t.rearrange("p (c k) -> p c k", k=128)
                        for j in range(WC):
                            wl = ch * WC + j
                            nc.tensor.matmul(
                                s_t[:, j * 128 : j * 128 + H],
                                k_cols[:, wl, :], q_cols[:, wl, :],
                                start=True, stop=True,
                            )
                        nc.scalar.activation(
                            out=es_t[:, ch * WC * H : (ch + 1) * WC * H].rearrange(
                                "p (c h) -> p c h", h=H
                            ),
                            in_=s3[:, :, 0:H], func=AF.Exp,
                        )
                        for j in range(WC):
                            wl = ch * WC + j
                            nc.tensor.matmul(
                                s_t[:, j * 128 + H : j * 128 + H + 1],
                                es_t[:, wl * H : (wl + 1) * H], invg_t,
                                start=True, stop=True,
                            )
                        nc.vector.reciprocal(
                            out=rt_t[:, ch * WC : (ch + 1) * WC], in_=s3[:, :, H]
                        )
                        for pair in range(2):
                            vp = ps.tile([H, 512], F32, tag="vtp", bufs=1)
                            for j2 in range(2):
                                wl = ch * WC + pair * 2 + j2
                                for ci in range(2):
                                    nc.tensor.matmul(
                                        vp[:, j2 * 256 : (j2 + 1) * 256],
                                        x_cols[ci][:, half * WH + wl, :],
                                        wvt_t[:, ci, :],
                                        start=(ci == 0), stop=(ci == 1),
                                    )
                            nc.scalar.copy(
                                out=vt_t[:, ch * WC + pair * 2 : ch * WC + pair * 2 + 2, :],
                                in_=vp,
                            )

                    # ---- B2: gamma/colsum as bf16 for the bcast matmul --------
                    rtb_t = rtp.tile([H, WH], BF16, tag="rw")
                    nc.vector.tensor_copy(rtb_t, rt_t)

                    # ---- B3: bcast, U, normalize, residual --------------------
                    for ch in range(NCH):
                        bcp = ps.tile([P, WC * H], F32, tag="bcp", bufs=2)
                        for j in range(WC):
                            wl = ch * WC + j
                            nc.tensor.matmul(
                                bcp[:, j * H : (j + 1) * H],
                                rtb_t[:, wl : wl + 1].to_broadcast([H, P]),
                                idb_t, start=True, stop=True,
                            )
                        bc_t = bctp.tile([P, WC * H], BF16, tag="bc")
                        nc.vector.tensor_copy(bc_t, bcp)
                        bc3 = bc_t.rearrange("p (c h) -> p c h", h=H)
                        for ci in range(2):
                            u_t = ps.tile([P, 512], F32, tag="u", bufs=3)
                            u3 = u_t.rearrange("p (c k) -> p c k", k=128)
                            for j in range(WC):
                                wl = ch * WC + j
                                nc.tensor.matmul(
                                    u_t[:, j * 128 : j * 128 + H],
                                    vt_t[:, wl, ci * 128 : (ci + 1) * 128],
                                    es_t[:, wl * H : (wl + 1) * H],
                                    start=True, stop=True,
                                )
                            t_t = ttp.tile([P, WC * H], BF16, tag="t")
                            t3 = t_t.rearrange("p (c h) -> p c h", h=H)
                            nc.vector.tensor_mul(t3, u3[:, :, 0:H], bc3)
                            xslice = x_cols[ci][
                                :, half * WH + ch * WC : half * WH + (ch + 1) * WC, :
                            ]
                            fslice = f_cols[ci][
                                :, half * WH + ch * WC : half * WH + (ch + 1) * WC, :
                            ]
                            eng = getattr(nc, TTADD_ENGINE)
                            eng.tensor_add(fslice, t3, xslice)

                for ci in range(2):
                    nc.sync.dma_start(out=out_d.ap()[b, ci], in_=f_ts[ci])
    nc.compile()
    return nc


_NC_CACHE = None


def _get_nc():
    global _NC_CACHE
    if _NC_CACHE is None:
        _NC_CACHE = _build()
    return _NC_CACHE


def kernel(x, Wq, bq, Wk, bk, Wv, bv, gamma):
    x = np.asarray(x, np.float32)
    Wq = np.asarray(Wq, np.float32)
    bq = np.asarray(bq, np.float32)
    Wk = np.asarray(Wk, np.float32)
    bk = np.asarray(bk, np.float32)
    Wv = np.asarray(Wv, np.float32)
    bv = np.asarray(bv, np.float32)
    g = float(np.asarray(gamma, np.float32)[0])

    C = 256
    e = (g * np.linalg.solve(np.eye(C, dtype=np.float64) + g * Wv.astype(np.float64),
                             bv.astype(np.float64))).astype(np.float32)
    xb = (x + e[None, :, None, None]).astype(ml_dtypes.bfloat16)
    xb = np.ascontiguousarray(xb).reshape(16, 2, P, HW)

    cblob = np.zeros((P, 866), np.float32)
    cblob[:, 0:128] = np.stack([Wq[:, :128].T, Wq[:, 128:].T], axis=1).reshape(P, 128)
    cblob[:, 128:256] = np.stack([Wk[:, :128].T, Wk[:, 128:].T], axis=1).reshape(P, 128)
    cblob[:, 256:768] = np.stack([Wv[:, :128].T, Wv[:, 128:].T], axis=1).reshape(P, 512)
    cblob[0:H, 769] = 1.0 / g
    cblob[0:H, 770:866] = np.eye(H, dtype=np.float32)
    cblob = cblob.astype(ml_dtypes.bfloat16)
    bblob = np.zeros((P, 1), np.float32)
    bblob[0:64, 0] = bq - Wq @ e
    bblob[64:128, 0] = bk - Wk @ e

    nc = _get_nc()
    in_maps = []
    for core in range(8):
        in_maps.append({
            "xb": xb[core * B_LOC : (core + 1) * B_LOC],
            "cblob": cblob, "bblob": bblob,
        })
    res = run_bass_kernel_spmd(nc, in_maps, core_ids=list(range(8)))
    outs = [r["out"].reshape(B_LOC, C, H, W) for r in res.results]
    return np.concatenate(outs, axis=0)


def prepared_in_maps(inputs):
    """test-harness helper: the per-core in_maps for a full input dict."""
    import inspect
    sig = ("x", "Wq", "bq", "Wk", "bk", "Wv", "bv", "gamma")
    global _CAPTURE
    _CAPTURE = None
    # rebuild the same host prep by calling kernel body up to run: duplicate code
    x = np.asarray(inputs["x"], np.float32)
    Wq = np.asarray(inputs["Wq"], np.float32); bq = np.asarray(inputs["bq"], np.float32)
    Wk = np.asarray(inputs["Wk"], np.float32); bk = np.asarray(inputs["bk"], np.float32)
    Wv = np.asarray(inputs["Wv"], np.float32); bv = np.asarray(inputs["bv"], np.float32)
    g = float(np.asarray(inputs["gamma"], np.float32)[0])
    C = 256
    e = (g * np.linalg.solve(np.eye(C, dtype=np.float64) + g * Wv.astype(np.float64),
                             bv.astype(np.float64))).astype(np.float32)
    xb = (x + e[None, :, None, None]).astype(ml_dtypes.bfloat16)
    xb = np.ascontiguousarray(xb).reshape(16, 2, P, HW)
    cblob = np.zeros((P, 866), np.float32)
    cblob[:, 0:128] = np.stack([Wq[:, :128].T, Wq[:, 128:].T], axis=1).reshape(P, 128)
    cblob[:, 128:256] = np.stack([Wk[:, :128].T, Wk[:, 128:].T], axis=1).reshape(P, 128)
    cblob[:, 256:768] = np.stack([Wv[:, :128].T, Wv[:, 128:].T], axis=1).reshape(P, 512)
    cblob[0:H, 769] = 1.0 / g
    cblob[0:H, 770:866] = np.eye(H, dtype=np.float32)
    cblob = cblob.astype(ml_dtypes.bfloat16)
    bblob = np.zeros((P, 1), np.float32)
    bblob[0:64, 0] = bq - Wq @ e
    bblob[64:128, 0] = bk - Wk @ e
    return [
        {"xb": xb[c * B_LOC : (c + 1) * B_LOC], "cblob": cblob, "bblob": bblob}
        for c in range(8)
    ]


# revision 18
# speedup vs baseline: 1.0163x; 1.0163x over previous
"""ColAttention TRN2 kernel: 8-core data-parallel over batch (2 batches/core).

Math (per batch b, width-column w):
  Q = Wq@x+bq; K = Wk@x+bk; V = Wv@x+bv        (1x1 convs over c)
  S[h,g] = sum_q Q[q,h]K[q,g]; attn = softmax_g(S)
  out = gamma * (V @ attn^T) + x

Device pipeline (all matmuls bf16, fp32 PSUM accumulation):
  host folds bv/gamma*bv via e = gamma*(I+gamma*Wv)^-1 bv:  xb = x+e,
  bq' = bq-Wq@e, bk' = bk-Wk@e  =>  device never touches bv and the
  residual add of xb reproduces the reference exactly (algebra in notes).
  A : Q,K projections (batched over h*w, N=384 chunks)
  B1: per column: S^T[g,h] via MM(lhsT=K_col, rhs=Q_col); exp via ACT
      (no max-subtraction: |S|<~50 so exp stays finite in f32);
      colsum^T[h] via MM(lhsT=expS_col, rhs=ones); 1/colsum on DVE;
      V^T[g,c] via MM(lhsT=x_col, rhs=Wv^T)
  B2: transpose recip^T[h,w-half] -> recipW[w,h] on PE
  B3: bcast tile gamma*r[h] over 128 partitions via K=1 outer-product MM;
      U[c,h] = MM(lhsT=V^T, rhs=expS); final = U*bcast + xb; DMA out.
"""
import sys

sys.path.insert(0, "/opt/trn_rl_repo")

import numpy as np
import ml_dtypes

import concourse.bass as bass
import concourse.bacc as bacc
import concourse.mybir as mybir
import concourse.tile as tile
from concourse.bass_utils import run_bass_kernel_spmd

F32 = mybir.dt.float32
BF16 = mybir.dt.bfloat16
AF = mybir.ActivationFunctionType

P = 128
H = 96          # height = attention sequence length
W = 96          # width  = independent columns
HW = H * W
B_LOC = 2       # batches per core
WH = 48         # columns per w-half
WC = 4          # columns per B-chunk
NCH = WH // WC  # 12 chunks per w-half

# engines for the Q/K projection evacuation, round-robined
_QK_EVAC = ("vector",)
# engine for the final residual add
TTADD_ENGINE = "vector"


def _build():
    nc = bacc.Bacc("TRN2", target_bir_lowering=False, debug=False)

    xb_d = nc.dram_tensor("xb", [B_LOC, 2, P, HW], BF16, kind="ExternalInput")
    cb_d = nc.dram_tensor("cblob", [P, 866], BF16, kind="ExternalInput")
    bb_d = nc.dram_tensor("bblob", [P, 2], F32, kind="ExternalInput")
    out_d = nc.dram_tensor("out", [B_LOC, 2, P, HW], F32, kind="ExternalOutput")

    with tile.TileContext(nc) as tc:
        import contextlib

        ctx = contextlib.ExitStack()
        with ctx:
            consts = ctx.enter_context(tc.tile_pool(name="consts", bufs=1))
            xp = ctx.enter_context(tc.tile_pool(name="xp", bufs=1))
            qkp = ctx.enter_context(tc.tile_pool(name="qkp", bufs=2))
            esp = ctx.enter_context(tc.tile_pool(name="esp", bufs=2))
            vtp = ctx.enter_context(tc.tile_pool(name="vtp", bufs=1))
            rtp = ctx.enter_context(tc.tile_pool(name="rtp", bufs=2))
            fp = ctx.enter_context(tc.tile_pool(name="fp", bufs=1))
            bctp = ctx.enter_context(tc.tile_pool(name="bctp", bufs=3))
            ttp = ctx.enter_context(tc.tile_pool(name="ttp", bufs=3))
            ps = ctx.enter_context(tc.tile_pool(name="ps", bufs=2, space="PSUM"))

            cb_t = consts.tile([P, 866], BF16)
            bb_t = consts.tile([P, 2], F32)
            nc.sync.dma_start(out=cb_t, in_=cb_d.ap())
            nc.sync.dma_start(out=bb_t, in_=bb_d.ap())
            # observers: funnel DMA deps into one engine each (this walrus
            # accepts a single semaphore wait per instruction)
            nc.tensor.ldweights(cb_t[:, 0:128])
            bias_t = consts.tile([P, 2], F32)
            nc.vector.tensor_copy(bias_t, bb_t)
            wq_t = cb_t[:, 0:128].rearrange("p (c m) -> p c m", c=2)
            wk_t = cb_t[:, 128:256].rearrange("p (c m) -> p c m", c=2)
            wvt_t = cb_t[:, 256:768].rearrange("p (c m) -> p c m", c=2)
            bq_t = bias_t[0:64, 0:1]
            bk_t = bias_t[64:128, 0:1]
            gvec_t = bias_t[0:H, 1:2]
            invg_t = cb_t[0:H, 769:770]
            idb_t = cb_t[0:H, 770:866]

            for b in range(B_LOC):
                x_cm = xp.tile([P, 2, HW], BF16, tag="xcm")
                for ci in range(2):
                    nc.sync.dma_start(out=x_cm[:, ci, :], in_=xb_d.ap()[b, ci])
                # h-major views (x_cm holds h-major data in this variant)
                x_cols = [
                    x_cm[:, ci, :].rearrange("p (h w) -> p w h", w=W) for ci in range(2)
                ]
                x_rows = [
                    x_cm[:, ci, :].rearrange("p (h w) -> p h w", w=W) for ci in range(2)
                ]
                f_ts = [fp.tile([P, HW], F32, tag=f"f{ci}", name=f"f{ci}") for ci in range(2)]
                for f in f_ts:
                    nc.vector.memset(f[0:1, 0:1], 0.0)
                f_cols = [f.rearrange("p (h w) -> p w h", w=W) for f in f_ts]

                for half in range(2):
                    # ---- A: Q/K projections for this w-half -------------------
                    q_t = qkp.tile([64, WH * H], BF16, tag="q", bufs=1)
                    k_t = qkp.tile([64, WH * H], BF16, tag="k", bufs=1)
                    ei = 0
                    for (w_l, b_l, o_t) in ((wq_t, bq_t, q_t), (wk_t, bk_t, k_t)):
                        for hc in range(12):  # 8 h-rows x 48 cols = N=384
                            pr = ps.tile([64, 384], F32, tag="s", bufs=1)
                            for ci in range(2):
                                rhs = x_rows[ci][
                                    :, hc * 8 : (hc + 1) * 8, half * WH : (half + 1) * WH
                                ]
                                nc.tensor.matmul(
                                    pr, w_l[:, ci, :], rhs,
                                    start=(ci == 0), stop=(ci == 1),
                                )
                            dst = o_t[:, hc * 384 : (hc + 1) * 384]
                            if _QK_EVAC[ei % len(_QK_EVAC)] == "act":
                                nc.scalar.activation(
                                    out=dst, in_=pr, func=AF.Identity, bias=b_l, scale=1.0
                                )
                            else:
                                nc.vector.tensor_scalar(
                                    out=dst, in0=pr, scalar1=b_l, scalar2=None,
                                    op0=mybir.AluOpType.add,
                                )
                            ei += 1
                    q_cols = q_t.rearrange("p (h w) -> p w h", w=WH)
                    k_cols = k_t.rearrange("p (h w) -> p w h", w=WH)

                    # ---- B1: scores/exp/colsum/recip + V^T --------------------
                    es_t = esp.tile([H, WH * H], BF16, tag="es", bufs=1)
                    vt_t = vtp.tile([H, WH, 256], BF16, tag="vt")
                    rt_t = rtp.tile([H, WH], F32, tag="rt")
                    for ch in range(NCH):
                        s_t = ps.tile([H, WC * H], F32, tag="s", bufs=1)
                        for j in range(WC):
                            wl = ch * WC + j
                            nc.tensor.matmul(
                                s_t[:, j * H : (j + 1) * H],
                                k_cols[:, wl, :], q_cols[:, wl, :],
                                start=True, stop=True,
                            )
                        es_ch = es_t[:, ch * WC * H : (ch + 1) * WC * H]
                        nc.scalar.activation(out=es_ch, in_=s_t[:, :], func=AF.Exp)
                        cs_p = ps.tile([H, WC], F32, tag="cs", bufs=1)
                        for j in range(WC):
                            wl = ch * WC + j
                            nc.tensor.matmul(
                                cs_p[:, j : j + 1],
                                es_t[:, wl * H : (wl + 1) * H], invg_t,
                                start=True, stop=True,
                            )
                        nc.vector.reciprocal(
                            out=rt_t[:, ch * WC : (ch + 1) * WC], in_=cs_p
                        )
                        for pair in range(2):
                            vp = ps.tile([H, 512], F32, tag="vtp", bufs=1)
                            for j2 in range(2):
                                wl = ch * WC + pair * 2 + j2
                                for ci in range(2):
                                    nc.tensor.matmul(
                                        vp[:, j2 * 256 : (j2 + 1) * 256],
                                        x_cols[ci][:, half * WH + wl, :],
                                        wvt_t[:, ci, :],
                                        start=(ci == 0), stop=(ci == 1),
                                    )
                            nc.scalar.copy(
                                out=vt_t[:, ch * WC + pair * 2 : ch * WC + pair * 2 + 2, :],
                                in_=vp,
                            )

                    # ---- B2: gamma/colsum as bf16 for the bcast matmul --------
                    rtb_t = rtp.tile([H, WH], BF16, tag="rw")
                    nc.vector.tensor_copy(rtb_t, rt_t)

                    # ---- B3: bcast, U, normalize, residual --------------------
                    for ch in range(NCH):
                        bcp = ps.tile([P, WC * H], F32, tag="bcp", bufs=2)
                        for j in range(WC):
                            wl = ch * WC + j
                            nc.tensor.matmul(
                                bcp[:, j * H : (j + 1) * H],
                                rtb_t[:, wl : wl + 1].to_broadcast([H, P]),
                                idb_t, start=True, stop=True,
                            )
                        bc_t = bctp.tile([P, WC * H], BF16, tag="bc")
                        nc.vector.tensor_copy(bc_t, bcp)
                        bc3 = bc_t.rearrange("p (c h) -> p c h", h=H)
                        for ci in range(2):
                            u_t = ps.tile([P, WC * H], F32, tag="u", bufs=3)
                            u3 = u_t.rearrange("p (c k) -> p c k", k=H)
                            for j in range(WC):
                                wl = ch * WC + j
                                nc.tensor.matmul(
                                    u_t[:, j * H : (j + 1) * H],
                                    vt_t[:, wl, ci * 128 : (ci + 1) * 128],
                                    es_t[:, wl * H : (wl + 1) * H],
                                    start=True, stop=True,
                                )
                            t_t = ttp.tile([P, WC * H], BF16, tag="t")
                            t3 = t_t.rearrange("p (c h) -> p c h", h=H)
                            nc.vector.tensor_mul(t3, u3, bc3)
                            xslice = x_cols[ci][
                                :, half * WH + ch * WC : half * WH + (ch + 1) * WC, :
                            ]
                            fslice = f_cols[ci][
                                :, half * WH + ch * WC : half * WH + (ch + 1) * WC, :
                            ]
                            eng = getattr(nc, TTADD_ENGINE)
                            eng.tensor_add(fslice, t3, xslice)

                for ci in range(2):
                    nc.sync.dma_start(out=out_d.ap()[b, ci], in_=f_ts[ci])
    nc.compile()
    return nc


_NC_CACHE = None


def _get_nc():
    global _NC_CACHE
    if _NC_CACHE is None:
        _NC_CACHE = _build()
    return _NC_CACHE


def kernel(x, Wq, bq, Wk, bk, Wv, bv, gamma):
    x = np.asarray(x, np.float32)
    Wq = np.asarray(Wq, np.float32)
    bq = np.asarray(bq, np.float32)
    Wk = np.asarray(Wk, np.float32)
    bk = np.asarray(bk, np.float32)
    Wv = np.asarray(Wv, np.float32)
    bv = np.asarray(bv, np.float32)
    g = float(np.asarray(gamma, np.float32)[0])

    C = 256
    e = (g * np.linalg.solve(np.eye(C, dtype=np.float64) + g * Wv.astype(np.float64),
                             bv.astype(np.float64))).astype(np.float32)
    xb = (x + e[None, :, None, None]).astype(ml_dtypes.bfloat16)
    xb = np.ascontiguousarray(xb).reshape(16, 2, P, HW)

    cblob = np.zeros((P, 866), np.float32)
    cblob[:, 0:128] = np.stack([Wq[:, :128].T, Wq[:, 128:].T], axis=1).reshape(P, 128)
    cblob[:, 128:256] = np.stack([Wk[:, :128].T, Wk[:, 128:].T], axis=1).reshape(P, 128)
    cblob[:, 256:768] = np.stack([Wv[:, :128].T, Wv[:, 128:].T], axis=1).reshape(P, 512)
    cblob[0:H, 769] = 1.0 / g
    cblob[0:H, 770:866] = np.eye(H, dtype=np.float32)
    cblob = cblob.astype(ml_dtypes.bfloat16)
    bblob = np.zeros((P, 2), np.float32)
    bblob[0:64, 0] = bq - Wq @ e
    bblob[64:128, 0] = bk - Wk @ e
    bblob[0:H, 1] = g

    nc = _get_nc()
    in_maps = []
    for core in range(8):
        in_maps.append({
            "xb": xb[core * B_LOC : (core + 1) * B_LOC],
            "cblob": cblob, "bblob": bblob,
        })
    res = run_bass_kernel_spmd(nc, in_maps, core_ids=list(range(8)))
    outs = [r["out"].reshape(B_LOC, C, H, W) for r in res.results]
    return np.concatenate(outs, axis=0)


def prepared_in_maps(inputs):
    """test-harness helper: the per-core in_maps for a full input dict."""
    import inspect
    sig = ("x", "Wq", "bq", "Wk", "bk", "Wv", "bv", "gamma")
    global _CAPTURE
    _CAPTURE = None
    # rebuild the same host prep by calling kernel body up to run: duplicate code
    x = np.asarray(inputs["x"], np.float32)
    Wq = np.asarray(inputs["Wq"], np.float32); bq = np.asarray(inputs["bq"], np.float32)
    Wk = np.asarray(inputs["Wk"], np.float32); bk = np.asarray(inputs["bk"], np.float32)
    Wv = np.asarray(inputs["Wv"], np.float32); bv = np.asarray(inputs["bv"], np.float32)
    g = float(np.asarray(inputs["gamma"], np.float32)[0])
    C = 256
    e = (g * np.linalg.solve(np.eye(C, dtype=np.float64) + g * Wv.astype(np.float64),
                             bv.astype(np.float64))).astype(np.float32)
    xb = (x + e[None, :, None, None]).astype(ml_dtypes.bfloat16)
    xb = np.ascontiguousarray(xb).reshape(16, 2, P, HW)
    cblob = np.zeros((P, 866), np.float32)
    cblob[:, 0:128] = np.stack([Wq[:, :128].T, Wq[:, 128:].T], axis=1).reshape(P, 128)
    cblob[:, 128:256] = np.stack([Wk[:, :128].T, Wk[:, 128:].T], axis=1).reshape(P, 128)
    cblob[:, 256:768] = np.stack([Wv[:, :128].T, Wv[:, 128:].T], axis=1).reshape(P, 512)
    cblob[0:H, 769] = 1.0 / g
    cblob[0:H, 770:866] = np.eye(H, dtype=np.float32)
    cblob = cblob.astype(ml_dtypes.bfloat16)
    bblob = np.zeros((P, 2), np.float32)
    bblob[0:64, 0] = bq - Wq @ e
    bblob[64:128, 0] = bk - Wk @ e
    bblob[0:H, 1] = g
    return [
        {"xb": xb[c * B_LOC : (c + 1) * B_LOC], "cblob": cblob, "bblob": bblob}
        for c in range(8)
    ]
